# revision 1
# baseline (speedup 1.0000x reference)
"""DetectionLoss Trainium2 kernel.

Data-parallel over batch: B=16 split across 8 NeuronCores (2 batches/core).
Each core computes masked partial sums (cls_sum, box_sum, obj_sum, count)
over its 2x16x1000 predictions; host combines the 8 partial vectors and does
the final division.

Math notes (vs the jax reference):
- argmax_g iou(p,g) == argmax_g [ln(inter) - ln(ap+ag)] because
  iou = r/(1-r) with r = inter/(ap+ag) monotone increasing in r.
- max_iou > 0.5  <=>  max_g(ln inter - ln(ap+ag)) > ln(1/3).
- cls BCE term collapses: one-hot vs one-hot BCE mean over 80 classes is
  C0 + (pred_cls != matched_cls)/80 elementwise.
- matched-GT gather is done as a one-hot (is_ge vs row max) matrix times the
  GT attribute table on the TensorEngine (transpose + matmul).

Prediction index p is remapped p = r*8 + t (r: 125 partitions, t: 8 free
columns); all loss sums are permutation invariant so the remap is free.
"""

import sys

sys.path.insert(0, "/opt/trn_rl_repo")

import numpy as np

B, F, P, G = 16, 16, 1000, 100
NCORES = 8
BL = B // NCORES          # batches per core
BF = BL * F               # (b,f) pairs per core
R, T = 125, 8             # p = r*8 + t
NUM_CLASSES = 80

_LOG2 = 0.6931471805599453
_SP1 = 0.31326168751822286        # log1p(exp(-1))
C0 = (_SP1 + (NUM_CLASSES - 1) * _LOG2) / NUM_CLASSES
C1 = 1.0 / NUM_CLASSES
LN13 = float(np.log(np.float32(1.0) / np.float32(3.0)))

_CACHE = {}


def _build(mode="full"):
    mode_set = set(mode.split(","))
    import os
    import concourse.bass as bass
    import concourse.bacc as bacc
    import concourse.tile as tile
    from concourse import mybir
    from concourse.masks import make_identity

    f32 = mybir.dt.float32
    i32 = mybir.dt.int32
    Alu = mybir.AluOpType
    Act = mybir.ActivationFunctionType

    nc = bacc.Bacc(None)
    pb_d = nc.dram_tensor("pb", [BF, R, T, 4], f32, kind="ExternalInput")
    sc_d = nc.dram_tensor("sc", [BF, R, T], f32, kind="ExternalInput")
    pc_d = nc.dram_tensor("pc", [BF, R, T], i32, kind="ExternalInput")
    gtb_d = nc.dram_tensor("gtb", [BF, G, 4], f32, kind="ExternalInput")
    gtc_d = nc.dram_tensor("gtc", [BF, G], i32, kind="ExternalInput")
    out_d = nc.dram_tensor("partials", [4, 1], f32, kind="ExternalOutput")

    with tile.TileContext(nc) as tc:
        with (
            tc.tile_pool(name="st", bufs=1) as st,
            tc.tile_pool(name="pair", bufs=2) as pair,
            tc.tile_pool(name="s2", bufs=1) as s2,
            tc.tile_pool(name="ps_pl", bufs=1, space="PSUM") as ps_pl,
            tc.tile_pool(name="ps_tr", bufs=1, space="PSUM") as ps_tr,
            tc.tile_pool(name="ps_mg", bufs=2, space="PSUM") as ps_mg,
            tc.tile_pool(name="ps_fin", bufs=1, space="PSUM") as ps_fin,
        ):
            # ---- static setup -------------------------------------------
            ident = st.tile([128, 128], f32)
            make_identity(nc, ident[:])
            ones_row = st.tile([1, 128], f32)
            nc.vector.memset(ones_row[:], 1.0)
            ones_col = st.tile([128, 1], f32)
            nc.vector.memset(ones_col[:], 1.0)
            zero_b = st.tile([128, 1], f32)
            nc.vector.memset(zero_b[:], 0.0)
            one_b = st.tile([128, 1], f32)
            nc.vector.memset(one_b[:], 1.0)
            eps_b = st.tile([128, 1], f32)
            nc.vector.memset(eps_b[:], 1e-12)

            # ---- whole-core input loads ---------------------------------
            pb_all = st.tile([R, BF, T, 4], f32)
            sc_all = st.tile([R, BF, T], f32)
            pc_i = st.tile([R, BF, T], i32)
            nc.sync.dma_start(out=pb_all[:], in_=pb_d.rearrange("a r t c -> r a t c"))
            nc.sync.dma_start(out=sc_all[:], in_=sc_d.rearrange("a r t -> r a t"))
            nc.sync.dma_start(out=pc_i[:], in_=pc_d.rearrange("a r t -> r a t"))
            pc_all = st.tile([R, BF, T], f32)
            nc.vector.tensor_copy(out=pc_all[:], in_=pc_i[:])

            gtb_g = st.tile([G, BF, 4], f32)        # g on partitions
            nc.sync.dma_start(out=gtb_g[:], in_=gtb_d.rearrange("a g c -> g a c"))
            gtb_row = st.tile([BF, G, 4], f32)      # bf on partitions
            nc.sync.dma_start(out=gtb_row[:], in_=gtb_d[:])
            gtc_row_i = st.tile([BF, G], i32)
            nc.sync.dma_start(out=gtc_row_i[:], in_=gtc_d[:])

            # agcls_cat[bf, 0, g] = gt area, [bf, 1, g] = gt class (f32)
            agcls_cat = st.tile([BF, 2, G], f32)
            wg = st.tile([BF, G], f32)
            hg = st.tile([BF, G], f32)
            nc.vector.tensor_tensor(out=wg[:], in0=gtb_row[:, :, 2], in1=gtb_row[:, :, 0], op=Alu.subtract)
            nc.vector.tensor_tensor(out=hg[:], in0=gtb_row[:, :, 3], in1=gtb_row[:, :, 1], op=Alu.subtract)
            nc.vector.tensor_tensor(out=agcls_cat[:, 0, :], in0=wg[:], in1=hg[:], op=Alu.mult)
            nc.vector.tensor_copy(out=agcls_cat[:, 1, :], in_=gtc_row_i[:])

            # transpose ag and cls rows to [G, 2, BF] for the gather matmuls
            agclsT_ps = ps_pl.tile([G, 2, 128], f32)
            nc.tensor.transpose(agclsT_ps[:, 0, :BF], agcls_cat[:, 0, :], ident[:BF, :BF])
            nc.tensor.transpose(agclsT_ps[:, 1, :BF], agcls_cat[:, 1, :], ident[:BF, :BF])
            agclsT = st.tile([G, 2, 128], f32)
            nc.scalar.copy(agclsT[:, :, :BF], agclsT_ps[:, :, :BF])

            # per-(r,t) pred areas
            ap_all = st.tile([R, BF, T], f32)
            wp = st.tile([R, BF, T], f32)
            hp = st.tile([R, BF, T], f32)
            nc.vector.tensor_tensor(out=wp[:], in0=pb_all[:, :, :, 2], in1=pb_all[:, :, :, 0], op=Alu.subtract)
            nc.vector.tensor_tensor(out=hp[:], in0=pb_all[:, :, :, 3], in1=pb_all[:, :, :, 1], op=Alu.subtract)
            nc.vector.tensor_tensor(out=ap_all[:], in0=wp[:], in1=hp[:], op=Alu.mult)

            mask_all = st.tile([R, BF, T], f32)
            matched = st.tile([R, BF, T, 6], f32)

            if mode_set & {"nopair", "nogather"}:
                nc.vector.memset(matched[:], 1.0)
            if "nopair" in mode_set:
                nc.vector.memset(mask_all[:], 1.0)

            # ---- per-(b,f) pipeline -------------------------------------
            for bf in range(BF) if "nopair" not in mode_set else []:
                # stage this bf's GT rows at partition 0 (matmul operands must
                # start at partition 0/32/64), then broadcast along partitions
                # via a ones-column matmul
                rhs_box = pair.tile([1, G, 4], f32)
                nc.sync.dma_start(out=rhs_box[:], in_=gtb_d[bf])
                rhs_agcls = pair.tile([1, 2, G], f32)
                nc.sync.dma_start(out=rhs_agcls[:], in_=agcls_cat[bf : bf + 1, :, :])
                pl_ps = ps_pl.tile([R, G, 4], f32)
                nc.tensor.matmul(pl_ps[:], ones_row[0:1, :R], rhs_box[:])
                pl2_ps = ps_pl.tile([R, 2, G], f32)
                nc.tensor.matmul(pl2_ps[:], ones_row[0:1, :R], rhs_agcls[:])
                planes = pair.tile([R, 6, G], f32)  # x1,y1,x2,y2,ag,cls
                nc.scalar.copy(planes[:, 0:4, :], pl_ps.rearrange("r g c -> r c g"))
                nc.scalar.copy(planes[:, 4:6, :], pl2_ps[:])

                def pl_b(c):
                    return planes[:, c, :].unsqueeze(1).broadcast_to([R, T, G])

                def pb_b(c):
                    return pb_all[:, bf, :, c].unsqueeze(2).broadcast_to([R, T, G])

                t1x = pair.tile([R, T, G], f32)
                t2x = pair.tile([R, T, G], f32)
                wx = pair.tile([R, T, G], f32)
                rx = pair.tile([R, T, G], f32)
                t1y = pair.tile([R, T, G], f32)
                t2y = pair.tile([R, T, G], f32)
                wy = pair.tile([R, T, G], f32)
                ry = pair.tile([R, T, G], f32)
                inter = pair.tile([R, T, G], f32)
                apag = pair.tile([R, T, G], f32)
                li = pair.tile([R, T, G], f32)
                lc = pair.tile([R, T, G], f32)
                d = pair.tile([R, T, G], f32)
                maxd = pair.tile([R, T], f32)
                S = pair.tile([R, T, G], f32)

                nc.vector.tensor_tensor(out=t1x[:], in0=pl_b(0), in1=pb_b(0), op=Alu.max)
                nc.vector.tensor_tensor(out=t2x[:], in0=pl_b(2), in1=pb_b(2), op=Alu.min)
                nc.gpsimd.tensor_tensor(out=wx[:], in0=t2x[:], in1=t1x[:], op=Alu.subtract)
                nc.scalar.activation(out=rx[:], in_=wx[:], func=Act.Relu, bias=zero_b[:R], scale=1.0)
                nc.vector.tensor_tensor(out=t1y[:], in0=pl_b(1), in1=pb_b(1), op=Alu.max)
                nc.vector.tensor_tensor(out=t2y[:], in0=pl_b(3), in1=pb_b(3), op=Alu.min)
                nc.gpsimd.tensor_tensor(out=wy[:], in0=t2y[:], in1=t1y[:], op=Alu.subtract)
                nc.scalar.activation(out=ry[:], in_=wy[:], func=Act.Relu, bias=zero_b[:R], scale=1.0)
                nc.vector.tensor_tensor(out=inter[:], in0=rx[:], in1=ry[:], op=Alu.mult)
                ap_b = ap_all[:, bf, :].unsqueeze(2).broadcast_to([R, T, G])
                nc.vector.tensor_tensor(out=apag[:], in0=pl_b(4), in1=ap_b, op=Alu.add)
                nc.scalar.activation(out=li[:], in_=inter[:], func=Act.Ln, bias=eps_b[:R], scale=1.0)
                nc.scalar.activation(out=lc[:], in_=apag[:], func=Act.Ln, bias=zero_b[:R], scale=1.0)
                nc.gpsimd.tensor_tensor(out=d[:], in0=li[:], in1=lc[:], op=Alu.subtract)
                nc.vector.tensor_reduce(out=maxd[:], in_=d[:], axis=mybir.AxisListType.X, op=Alu.max)
                nc.vector.tensor_scalar(
                    out=mask_all[:, bf, :], in0=maxd[:], scalar1=LN13, scalar2=None, op0=Alu.is_gt
                )
                maxd_b = maxd.unsqueeze(2).broadcast_to([R, T, G])
                nc.vector.tensor_tensor(out=S[:], in0=d[:], in1=maxd_b, op=Alu.is_ge)

                if "nogather" in mode_set:
                    continue
                # gather matched GT attrs: transpose S per t, then matmul
                st_ps = ps_tr.tile([G, T, 128], f32)
                st_sb = pair.tile([G, T, 128], f32)
                for t in range(T):
                    nc.tensor.transpose(st_ps[:, t, :R], S[:, t, :], ident[:R, :R])
                nc.scalar.copy(st_sb[:, :, :R], st_ps[:, :, :R])
                mg_ps = ps_mg.tile([R, T, 6], f32)
                for t in range(T):
                    nc.tensor.matmul(mg_ps[:, t, 0:4], st_sb[:, t, :R], gtb_g[:, bf, :])
                    nc.tensor.matmul(mg_ps[:, t, 4:6], st_sb[:, t, :R], agclsT[:, :, bf])
                nc.scalar.copy(matched[:, bf, :, :], mg_ps[:])

            # ---- stage 2: elementwise GIoU/cls/obj + masked sums --------
            def pbc(c):
                return pb_all[:, :, :, c]

            def mgc(c):
                return matched[:, :, :, c]

            sh = [R, BF, T]
            ltx = s2.tile(sh, f32)
            lty = s2.tile(sh, f32)
            rbx = s2.tile(sh, f32)
            rby = s2.tile(sh, f32)
            wx2 = s2.tile(sh, f32)
            wy2 = s2.tile(sh, f32)
            rx2 = s2.tile(sh, f32)
            ry2 = s2.tile(sh, f32)
            inter2 = s2.tile(sh, f32)
            u1 = s2.tile(sh, f32)
            union2 = s2.tile(sh, f32)
            elx = s2.tile(sh, f32)
            ely = s2.tile(sh, f32)
            erx = s2.tile(sh, f32)
            ery = s2.tile(sh, f32)
            ew = s2.tile(sh, f32)
            eh = s2.tile(sh, f32)
            earea = s2.tile(sh, f32)
            ru = s2.tile(sh, f32)
            re_ = s2.tile(sh, f32)
            iou2 = s2.tile(sh, f32)
            esu = s2.tile(sh, f32)
            t3 = s2.tile(sh, f32)
            b1 = s2.tile(sh, f32)
            box_per = s2.tile(sh, f32)
            eqc = s2.tile(sh, f32)
            cls_per = s2.tile(sh, f32)
            sabs = s2.tile(sh, f32)
            sexp = s2.tile(sh, f32)
            sln = s2.tile(sh, f32)
            srelu = s2.tile(sh, f32)
            obj_per = s2.tile(sh, f32)
            scratch = s2.tile(sh, f32)
            accs = s2.tile([R, 4], f32)

            if mode_set & {"min1", "min2", "s2half1"}:
                nc.vector.memset(accs[:], 1.0)
            if "min1" not in mode_set:
                nc.vector.tensor_tensor(out=ltx[:], in0=pbc(0), in1=mgc(0), op=Alu.max)
                nc.vector.tensor_tensor(out=lty[:], in0=pbc(1), in1=mgc(1), op=Alu.max)
                nc.vector.tensor_tensor(out=rbx[:], in0=pbc(2), in1=mgc(2), op=Alu.min)
                nc.vector.tensor_tensor(out=rby[:], in0=pbc(3), in1=mgc(3), op=Alu.min)
                nc.vector.tensor_tensor(out=wx2[:], in0=rbx[:], in1=ltx[:], op=Alu.subtract)
                nc.vector.tensor_tensor(out=wy2[:], in0=rby[:], in1=lty[:], op=Alu.subtract)
                nc.scalar.activation(out=rx2[:], in_=wx2[:], func=Act.Relu, bias=zero_b[:R], scale=1.0)
                nc.scalar.activation(out=ry2[:], in_=wy2[:], func=Act.Relu, bias=zero_b[:R], scale=1.0)
                nc.vector.tensor_tensor(out=inter2[:], in0=rx2[:], in1=ry2[:], op=Alu.mult)
                nc.vector.tensor_tensor(out=u1[:], in0=ap_all[:], in1=mgc(4), op=Alu.add)
                nc.vector.tensor_tensor(out=union2[:], in0=u1[:], in1=inter2[:], op=Alu.subtract)
                nc.vector.tensor_tensor(out=elx[:], in0=pbc(0), in1=mgc(0), op=Alu.min)
                nc.vector.tensor_tensor(out=ely[:], in0=pbc(1), in1=mgc(1), op=Alu.min)
                nc.vector.tensor_tensor(out=erx[:], in0=pbc(2), in1=mgc(2), op=Alu.max)
                nc.vector.tensor_tensor(out=ery[:], in0=pbc(3), in1=mgc(3), op=Alu.max)
                nc.gpsimd.tensor_tensor(out=ew[:], in0=erx[:], in1=elx[:], op=Alu.subtract)
                nc.gpsimd.tensor_tensor(out=eh[:], in0=ery[:], in1=ely[:], op=Alu.subtract)
                nc.vector.tensor_tensor(out=earea[:], in0=ew[:], in1=eh[:], op=Alu.mult)
                nc.vector.reciprocal(out=ru[:], in_=union2[:])
                nc.vector.reciprocal(out=re_[:], in_=earea[:])
                nc.vector.tensor_tensor(out=iou2[:], in0=inter2[:], in1=ru[:], op=Alu.mult)
                nc.vector.tensor_tensor(out=esu[:], in0=earea[:], in1=union2[:], op=Alu.subtract)
                nc.vector.tensor_tensor(out=t3[:], in0=esu[:], in1=re_[:], op=Alu.mult)
                nc.vector.tensor_tensor(out=b1[:], in0=t3[:], in1=iou2[:], op=Alu.subtract)
                nc.vector.tensor_scalar(out=box_per[:], in0=b1[:], scalar1=1.0, scalar2=None, op0=Alu.add)
                pass
            if "min1" not in mode_set and "s2half1" not in mode_set:
                nc.vector.tensor_tensor(out=eqc[:], in0=pc_all[:], in1=mgc(5), op=Alu.is_equal)
                nc.vector.tensor_scalar(
                    out=cls_per[:], in0=eqc[:], scalar1=-C1, scalar2=C0 + C1, op0=Alu.mult, op1=Alu.add
                )
                # obj: softplus(-s) = relu(-s) + ln(1 + exp(-|s|))
                nc.scalar.activation(out=sabs[:], in_=sc_all[:], func=Act.Abs, bias=zero_b[:R], scale=1.0)
                nc.scalar.activation(out=sexp[:], in_=sabs[:], func=Act.Exp, bias=zero_b[:R], scale=-1.0)
                nc.scalar.activation(out=sln[:], in_=sexp[:], func=Act.Ln, bias=one_b[:R], scale=1.0)
                nc.scalar.activation(out=srelu[:], in_=sc_all[:], func=Act.Relu, bias=zero_b[:R], scale=-1.0)
                nc.vector.tensor_tensor(out=obj_per[:], in0=sln[:], in1=srelu[:], op=Alu.add)
                # masked sums -> accs columns (plain mult + reduce; accum_out
                # variants of TS/TTR fail at runtime on this stack)
                nc.vector.tensor_tensor(out=scratch[:], in0=cls_per[:], in1=mask_all[:], op=Alu.mult)
                nc.vector.tensor_reduce(out=accs[:, 0:1], in_=scratch[:], axis=mybir.AxisListType.XY, op=Alu.add)
                nc.vector.tensor_tensor(out=box_per[:], in0=box_per[:], in1=mask_all[:], op=Alu.mult)
                nc.vector.tensor_reduce(out=accs[:, 1:2], in_=box_per[:], axis=mybir.AxisListType.XY, op=Alu.add)
                nc.vector.tensor_tensor(out=obj_per[:], in0=obj_per[:], in1=mask_all[:], op=Alu.mult)
                nc.vector.tensor_reduce(out=accs[:, 2:3], in_=obj_per[:], axis=mybir.AxisListType.XY, op=Alu.add)
                nc.vector.tensor_reduce(out=accs[:, 3:4], in_=mask_all[:], axis=mybir.AxisListType.XY, op=Alu.add)
            fin_ps = ps_fin.tile([4, 1], f32)
            nc.tensor.matmul(fin_ps[:], accs[:], ones_col[:R, :])
            fin_sb = s2.tile([4, 1], f32)
            nc.scalar.copy(fin_sb[:], fin_ps[:])
            nc.sync.dma_start(out=out_d[:], in_=fin_sb[:])

    nc.finalize()
    return nc


def _get_nc():
    import os
    mode = os.environ.get("KMODE", "full")
    if "nc" not in _CACHE:
        _CACHE["nc"] = _build(mode)
    return _CACHE["nc"]


def _make_in_maps(pred_boxes, pred_scores, pred_classes, gt_boxes, gt_classes):
    in_maps = []
    for c in range(NCORES):
        sl = slice(c * BL, (c + 1) * BL)
        in_maps.append({
            "pb": np.ascontiguousarray(pred_boxes[sl]).reshape(BF, R, T, 4),
            "sc": np.ascontiguousarray(pred_scores[sl]).reshape(BF, R, T),
            "pc": np.ascontiguousarray(pred_classes[sl]).reshape(BF, R, T),
            "gtb": np.ascontiguousarray(gt_boxes[sl]).reshape(BF, G, 4),
            "gtc": np.ascontiguousarray(gt_classes[sl]).reshape(BF, G),
        })
    return in_maps


def _combine(partials):
    tot = np.zeros(4, dtype=np.float32)
    for p in partials:
        tot += p.reshape(4).astype(np.float32)
    cls_s, box_s, obj_s, n = tot
    denom = np.float32(max(n, 1.0))
    if n > 0:
        cls_l = np.float32(cls_s / denom)
        box_l = np.float32(box_s / denom)
        obj_l = np.float32(obj_s / denom)
    else:
        cls_l = box_l = obj_l = np.float32(0.0)
    loss = np.float32(cls_l + box_l + obj_l)
    return np.stack([loss, cls_l, box_l, obj_l]).astype(np.float32)


def kernel(pred_boxes, pred_scores, pred_classes, gt_boxes, gt_classes):
    from concourse.bass_utils import run_bass_kernel_spmd

    nc = _get_nc()
    in_maps = _make_in_maps(pred_boxes, pred_scores, pred_classes, gt_boxes, gt_classes)
    res = run_bass_kernel_spmd(nc, in_maps, list(range(NCORES)))
    return _combine([res.results[c]["partials"] for c in range(NCORES)])



# revision 7
# speedup vs baseline: 1.1548x; 1.1548x over previous
"""DetectionLoss Trainium2 kernel (v2).

Data-parallel over batch: B=16 split across 8 NeuronCores (2 batches/core).
Each core computes masked partial sums (cls_sum, box_sum, obj_sum, count)
over its 2x16x1000 predictions; host combines the 8 partial vectors and does
the final division.

v2 changes vs v1 (469.8us baseline):
- Gather matmuls merged: one rhs [G,6] (coords+area+cls) per (bf,t) instead
  of two -> 8 matmuls/bf instead of 16; operands bf16 (one-hot S and the gt
  attribute table are exact / well-conditioned in bf16).
- IoU intersection path in bf16 (DVE 2x mode on contiguous tensor_tensor);
  the argmax score d = ln(inter) - ln(ap+ag) stays f32 so exact ties (which
  would double-gather and corrupt the matched-GT sums) remain measure-zero.
- relu as vector tensor_scalar max(x,0) (2x/4x mode) instead of scalar-engine
  activation (1x).
- GT planes materialized once per bf via ones-matmul into PSUM, then one cheap
  scalar copy to SBUF bf16; pred coordinate planes pre-split [R,BF,T] so all
  broadcast access patterns are 2-dim.
- Engine rebalance: gpsimd takes wx/d/t1y, scalar takes ln/copies + two pred
  plane broadcasts, vector keeps the rest.

Prediction index p is remapped p = r*8 + t (r: 125 partitions, t: 8 free
columns); all loss sums are permutation invariant so the remap is free.
"""

import sys

sys.path.insert(0, "/opt/trn_rl_repo")

import numpy as np

B, F, P, G = 16, 16, 1000, 100
NCORES = 8
BL = B // NCORES          # batches per core
BF = BL * F               # (b,f) pairs per core
R, T = 125, 8             # p = r*8 + t
NUM_CLASSES = 80

_LOG2 = 0.6931471805599453
_SP1 = 0.31326168751822286        # log1p(exp(-1))
C0 = (_SP1 + (NUM_CLASSES - 1) * _LOG2) / NUM_CLASSES
C1 = 1.0 / NUM_CLASSES
LN13 = float(np.log(np.float32(1.0) / np.float32(3.0)))

_CACHE = {}


def _build(mode="full"):
    mode_set = set(mode.split(","))
    import concourse.bass as bass
    import concourse.bacc as bacc
    import concourse.tile as tile
    from concourse import mybir
    from concourse.masks import make_identity

    f32 = mybir.dt.float32
    bf16 = mybir.dt.bfloat16
    i32 = mybir.dt.int32
    Alu = mybir.AluOpType
    Act = mybir.ActivationFunctionType

    nc = bacc.Bacc(None)
    pb_d = nc.dram_tensor("pb", [BF, R, T, 4], f32, kind="ExternalInput")
    sc_d = nc.dram_tensor("sc", [BF, R, T], f32, kind="ExternalInput")
    pc_d = nc.dram_tensor("pc", [BF, R, T], i32, kind="ExternalInput")
    gtb_d = nc.dram_tensor("gtb", [BF, G, 4], f32, kind="ExternalInput")
    gtc_d = nc.dram_tensor("gtc", [BF, G], i32, kind="ExternalInput")
    out_d = nc.dram_tensor("partials", [4, 1], f32, kind="ExternalOutput")

    with tile.TileContext(nc) as tc:
        with (
            tc.tile_pool(name="st", bufs=1) as st,
            tc.tile_pool(name="pair", bufs=2) as pair,
            tc.tile_pool(name="s2", bufs=1) as s2,
            tc.tile_pool(name="ps_pl", bufs=2, space="PSUM") as ps_pl,
            tc.tile_pool(name="ps_tr", bufs=1, space="PSUM") as ps_tr,
            tc.tile_pool(name="ps_mg", bufs=1, space="PSUM") as ps_mg,
        ):
            # ---- static setup -------------------------------------------
            ident_bf = st.tile([128, 128], bf16)
            make_identity(nc, ident_bf[:])
            ones_row_bf = st.tile([1, 128], bf16)
            nc.vector.memset(ones_row_bf[:], 1.0)
            ones_col = st.tile([128, 1], f32)
            nc.vector.memset(ones_col[:], 1.0)
            zero_b = st.tile([128, 1], f32)
            nc.vector.memset(zero_b[:], 0.0)
            one_b = st.tile([128, 1], f32)
            nc.vector.memset(one_b[:], 1.0)
            eps_b = st.tile([128, 1], f32)
            nc.vector.memset(eps_b[:], 1e-12)

            # ---- whole-core input loads ---------------------------------
            pb_all = st.tile([R, BF, T, 4], f32)
            sc_all = st.tile([R, BF, T], f32)
            pc_i = st.tile([R, BF, T], i32)
            nc.sync.dma_start(out=pb_all[:], in_=pb_d.rearrange("a r t c -> r a t c"))
            nc.sync.dma_start(out=sc_all[:], in_=sc_d.rearrange("a r t -> r a t"))
            nc.sync.dma_start(out=pc_i[:], in_=pc_d.rearrange("a r t -> r a t"))
            pc_all = st.tile([R, BF, T], f32)
            nc.vector.tensor_copy(out=pc_all[:], in_=pc_i[:])

            gtb_g = st.tile([G, BF, 4], f32)        # g on partitions
            nc.sync.dma_start(out=gtb_g[:], in_=gtb_d.rearrange("a g c -> g a c"))
            gtc_g_i = st.tile([G, BF], i32)
            nc.sync.dma_start(out=gtc_g_i[:], in_=gtc_d.rearrange("a g -> g a"))
            gtb_row = st.tile([BF, G, 4], f32)      # bf on partitions
            nc.sync.dma_start(out=gtb_row[:], in_=gtb_d[:])
            gtc_row_i = st.tile([BF, G], i32)
            nc.sync.dma_start(out=gtc_row_i[:], in_=gtc_d[:])

            # gt attribute tables:
            #   row layout [BF, 6, G] (bf16) -> per-bf broadcast-matmul rhs
            #   g layout   [G, BF, 6] (bf16) -> per-(bf,t) gather-matmul rhs
            # rows/cols: 0..3 = x1,y1,x2,y2; 4 = area; 5 = class
            gattr_row = st.tile([BF, 6, G], f32)
            for c in range(4):
                nc.vector.tensor_copy(out=gattr_row[:, c, :], in_=gtb_row[:, :, c])
            wg = st.tile([BF, G], f32)
            hg = st.tile([BF, G], f32)
            nc.vector.tensor_tensor(out=wg[:], in0=gtb_row[:, :, 2], in1=gtb_row[:, :, 0], op=Alu.subtract)
            nc.vector.tensor_tensor(out=hg[:], in0=gtb_row[:, :, 3], in1=gtb_row[:, :, 1], op=Alu.subtract)
            nc.vector.tensor_tensor(out=gattr_row[:, 4, :], in0=wg[:], in1=hg[:], op=Alu.mult)
            nc.vector.tensor_copy(out=gattr_row[:, 5, :], in_=gtc_row_i[:])
            gattr_row_bf = st.tile([BF, 6, G], bf16)
            nc.vector.tensor_copy(out=gattr_row_bf[:], in_=gattr_row[:])

            gattr_g = st.tile([G, BF, 6], f32)
            for c in range(4):
                nc.vector.tensor_copy(out=gattr_g[:, :, c], in_=gtb_g[:, :, c])
            wgg = st.tile([G, BF], f32)
            hgg = st.tile([G, BF], f32)
            nc.vector.tensor_tensor(out=wgg[:], in0=gtb_g[:, :, 2], in1=gtb_g[:, :, 0], op=Alu.subtract)
            nc.vector.tensor_tensor(out=hgg[:], in0=gtb_g[:, :, 3], in1=gtb_g[:, :, 1], op=Alu.subtract)
            nc.vector.tensor_tensor(out=gattr_g[:, :, 4], in0=wgg[:], in1=hgg[:], op=Alu.mult)
            nc.vector.tensor_copy(out=gattr_g[:, :, 5], in_=gtc_g_i[:])
            gattr_g_bf = st.tile([G, BF, 6], bf16)
            nc.vector.tensor_copy(out=gattr_g_bf[:], in_=gattr_g[:])

            # pred coordinate planes, contiguous [R, BF, T] each
            px = []
            for c in range(4):
                pxc = st.tile([R, BF, T], f32, name=f"px{c}")
                nc.vector.tensor_copy(out=pxc[:], in_=pb_all[:, :, :, c])
                px.append(pxc)
            ap_all = st.tile([R, BF, T], f32)
            wp = st.tile([R, BF, T], f32)
            hp = st.tile([R, BF, T], f32)
            nc.vector.tensor_tensor(out=wp[:], in0=px[2][:], in1=px[0][:], op=Alu.subtract)
            nc.gpsimd.tensor_tensor(out=hp[:], in0=px[3][:], in1=px[1][:], op=Alu.subtract)
            nc.vector.tensor_tensor(out=ap_all[:], in0=wp[:], in1=hp[:], op=Alu.mult)

            mask_all = st.tile([R, BF, T], f32)
            matched = st.tile([R, BF, T, 6], f32)

            if mode_set & {"nopair", "nogather"}:
                nc.vector.memset(matched[:], 1.0)
            if "nopair" in mode_set:
                nc.vector.memset(mask_all[:], 1.0)

            # ---- per-(b,f) pipeline -------------------------------------
            for bf in range(BF) if "nopair" not in mode_set else []:
                # stage this bf's gt attr rows at partition 0, broadcast to
                # [R, ., G] planes via ones-matmul (two PSUM tiles: 4+2 rows)
                rhs_pl = pair.tile([1, 6, G], bf16)
                nc.sync.dma_start(out=rhs_pl[:], in_=gattr_row_bf[bf : bf + 1, :, :])
                pl_ps = ps_pl.tile([R, 4, G], f32)
                nc.tensor.matmul(pl_ps[:], ones_row_bf[0:1, :R], rhs_pl[:, 0:4, :])
                pl2_ps = ps_pl.tile([R, 2, G], f32)
                nc.tensor.matmul(pl2_ps[:], ones_row_bf[0:1, :R], rhs_pl[:, 4:6, :])
                planes = pair.tile([R, 6, G], bf16)  # x1,y1,x2,y2,ag,cls
                nc.scalar.copy(planes[:, 0:4, :], pl_ps[:])
                nc.scalar.copy(planes[:, 4:6, :], pl2_ps[:])

                def pl_b(c):
                    return planes[:, c, :].unsqueeze(1).broadcast_to([R, T, G])

                def px_b(c):
                    return px[c][:, bf, :].unsqueeze(2).broadcast_to([R, T, G])

                # materialize the two x pred planes on the scalar engine so
                # t1x/t2x run as clean contiguous bf16 TTs on the DVE
                px1_b = pair.tile([R, T, G], bf16)
                nc.scalar.copy(px1_b[:], px_b(0))
                px2_b = pair.tile([R, T, G], bf16)
                nc.scalar.copy(px2_b[:], px_b(2))

                t1x = pair.tile([R, T, G], bf16)
                t2x = pair.tile([R, T, G], bf16)
                t1y = pair.tile([R, T, G], bf16)
                t2y = pair.tile([R, T, G], bf16)
                wx = pair.tile([R, T, G], bf16)
                wy = pair.tile([R, T, G], bf16)
                rx = pair.tile([R, T, G], bf16)
                ry = pair.tile([R, T, G], bf16)
                inter = pair.tile([R, T, G], bf16)
                apag = pair.tile([R, T, G], bf16)
                li = pair.tile([R, T, G], f32)
                lc = pair.tile([R, T, G], f32)
                d = pair.tile([R, T, G], f32)
                maxd = pair.tile([R, T], f32)
                S = pair.tile([R, T, G], bf16)

                nc.vector.tensor_tensor(out=t1x[:], in0=pl_b(0), in1=px1_b[:], op=Alu.max)
                nc.vector.tensor_tensor(out=t2x[:], in0=pl_b(2), in1=px2_b[:], op=Alu.min)
                nc.vector.tensor_tensor(out=t1y[:], in0=pl_b(1), in1=px_b(1), op=Alu.max)
                nc.vector.tensor_tensor(out=t2y[:], in0=pl_b(3), in1=px_b(3), op=Alu.min)
                nc.gpsimd.tensor_tensor(out=wx[:], in0=t2x[:], in1=t1x[:], op=Alu.subtract)
                nc.vector.tensor_tensor(out=wy[:], in0=t2y[:], in1=t1y[:], op=Alu.subtract)
                nc.vector.tensor_scalar(out=rx[:], in0=wx[:], scalar1=0.0, scalar2=None, op0=Alu.max)
                nc.vector.tensor_scalar(out=ry[:], in0=wy[:], scalar1=0.0, scalar2=None, op0=Alu.max)
                nc.vector.tensor_tensor(out=inter[:], in0=rx[:], in1=ry[:], op=Alu.mult)
                ap_b = ap_all[:, bf, :].unsqueeze(2).broadcast_to([R, T, G])
                nc.vector.tensor_tensor(out=apag[:], in0=pl_b(4), in1=ap_b, op=Alu.add)
                nc.scalar.activation(out=li[:], in_=inter[:], func=Act.Ln, bias=eps_b[:R], scale=1.0)
                nc.scalar.activation(out=lc[:], in_=apag[:], func=Act.Ln, bias=zero_b[:R], scale=1.0)
                nc.gpsimd.tensor_tensor(out=d[:], in0=li[:], in1=lc[:], op=Alu.subtract)
                nc.vector.tensor_reduce(out=maxd[:], in_=d[:], axis=mybir.AxisListType.X, op=Alu.max)
                nc.vector.tensor_scalar(
                    out=mask_all[:, bf, :], in0=maxd[:], scalar1=LN13, scalar2=None, op0=Alu.is_gt
                )
                maxd_b = maxd.unsqueeze(2).broadcast_to([R, T, G])
                nc.vector.tensor_tensor(out=S[:], in0=d[:], in1=maxd_b, op=Alu.is_ge)

                if "nogather" in mode_set:
                    continue
                # gather matched GT attrs: transpose S per t, then one
                # [G,R]x[G,6] matmul per t
                st_ps = ps_tr.tile([G, T, 128], bf16)
                st_sb = pair.tile([G, T, 128], bf16)
                for t in range(T):
                    nc.tensor.transpose(st_ps[:, t, :R], S[:, t, :], ident_bf[:R, :R])
                nc.scalar.copy(st_sb[:, :, :R], st_ps[:, :, :R])
                mg_ps = ps_mg.tile([R, T, 6], f32)
                for t in range(T):
                    nc.tensor.matmul(mg_ps[:, t, :], st_sb[:, t, :R], gattr_g_bf[:, bf, :])
                nc.scalar.copy(matched[:, bf, :, :], mg_ps[:])

            # ---- stage 2: elementwise GIoU/cls/obj + masked sums --------
            def pbc(c):
                return pb_all[:, :, :, c]

            def mgc(c):
                return matched[:, :, :, c]

            sh = [R, BF, T]
            ltx = s2.tile(sh, f32)
            lty = s2.tile(sh, f32)
            rbx = s2.tile(sh, f32)
            rby = s2.tile(sh, f32)
            wx2 = s2.tile(sh, f32)
            wy2 = s2.tile(sh, f32)
            rx2 = s2.tile(sh, f32)
            ry2 = s2.tile(sh, f32)
            inter2 = s2.tile(sh, f32)
            u1 = s2.tile(sh, f32)
            union2 = s2.tile(sh, f32)
            elx = s2.tile(sh, f32)
            ely = s2.tile(sh, f32)
            erx = s2.tile(sh, f32)
            ery = s2.tile(sh, f32)
            ew = s2.tile(sh, f32)
            eh = s2.tile(sh, f32)
            earea = s2.tile(sh, f32)
            ru = s2.tile(sh, f32)
            re_ = s2.tile(sh, f32)
            iou2 = s2.tile(sh, f32)
            esu = s2.tile(sh, f32)
            t3 = s2.tile(sh, f32)
            b1 = s2.tile(sh, f32)
            box_per = s2.tile(sh, f32)
            eqc = s2.tile(sh, f32)
            cls_per = s2.tile(sh, f32)
            sabs = s2.tile(sh, f32)
            sexp = s2.tile(sh, f32)
            sln = s2.tile(sh, f32)
            srelu = s2.tile(sh, f32)
            obj_per = s2.tile(sh, f32)
            scratch = s2.tile(sh, f32)
            accs = s2.tile([R, 4], f32)

            if mode_set & {"min1", "min2", "s2half1"}:
                nc.vector.memset(accs[:], 1.0)
            if "min1" not in mode_set:
                nc.vector.tensor_tensor(out=ltx[:], in0=pbc(0), in1=mgc(0), op=Alu.max)
                nc.vector.tensor_tensor(out=lty[:], in0=pbc(1), in1=mgc(1), op=Alu.max)
                nc.vector.tensor_tensor(out=rbx[:], in0=pbc(2), in1=mgc(2), op=Alu.min)
                nc.vector.tensor_tensor(out=rby[:], in0=pbc(3), in1=mgc(3), op=Alu.min)
                nc.gpsimd.tensor_tensor(out=wx2[:], in0=rbx[:], in1=ltx[:], op=Alu.subtract)
                nc.gpsimd.tensor_tensor(out=wy2[:], in0=rby[:], in1=lty[:], op=Alu.subtract)
                nc.scalar.activation(out=rx2[:], in_=wx2[:], func=Act.Relu, bias=zero_b[:R], scale=1.0)
                nc.scalar.activation(out=ry2[:], in_=wy2[:], func=Act.Relu, bias=zero_b[:R], scale=1.0)
                nc.vector.tensor_tensor(out=inter2[:], in0=rx2[:], in1=ry2[:], op=Alu.mult)
                nc.vector.tensor_tensor(out=u1[:], in0=ap_all[:], in1=mgc(4), op=Alu.add)
                nc.vector.tensor_tensor(out=union2[:], in0=u1[:], in1=inter2[:], op=Alu.subtract)
                nc.vector.tensor_tensor(out=elx[:], in0=pbc(0), in1=mgc(0), op=Alu.min)
                nc.vector.tensor_tensor(out=ely[:], in0=pbc(1), in1=mgc(1), op=Alu.min)
                nc.vector.tensor_tensor(out=erx[:], in0=pbc(2), in1=mgc(2), op=Alu.max)
                nc.vector.tensor_tensor(out=ery[:], in0=pbc(3), in1=mgc(3), op=Alu.max)
                nc.gpsimd.tensor_tensor(out=ew[:], in0=erx[:], in1=elx[:], op=Alu.subtract)
                nc.gpsimd.tensor_tensor(out=eh[:], in0=ery[:], in1=ely[:], op=Alu.subtract)
                nc.vector.tensor_tensor(out=earea[:], in0=ew[:], in1=eh[:], op=Alu.mult)
                nc.vector.reciprocal(out=ru[:], in_=union2[:])
                nc.vector.reciprocal(out=re_[:], in_=earea[:])
                nc.vector.tensor_tensor(out=iou2[:], in0=inter2[:], in1=ru[:], op=Alu.mult)
                nc.gpsimd.tensor_tensor(out=esu[:], in0=earea[:], in1=union2[:], op=Alu.subtract)
                nc.vector.tensor_tensor(out=t3[:], in0=esu[:], in1=re_[:], op=Alu.mult)
                nc.vector.tensor_tensor(out=b1[:], in0=t3[:], in1=iou2[:], op=Alu.subtract)
                nc.vector.tensor_scalar(out=box_per[:], in0=b1[:], scalar1=1.0, scalar2=None, op0=Alu.add)
                pass
            if "min1" not in mode_set and "s2half1" not in mode_set:
                nc.vector.tensor_tensor(out=eqc[:], in0=pc_all[:], in1=mgc(5), op=Alu.is_equal)
                nc.vector.tensor_scalar(
                    out=cls_per[:], in0=eqc[:], scalar1=-C1, scalar2=C0 + C1, op0=Alu.mult, op1=Alu.add
                )
                # obj: softplus(-s) = relu(-s) + ln(1 + exp(-|s|))
                nc.scalar.activation(out=sabs[:], in_=sc_all[:], func=Act.Abs, bias=zero_b[:R], scale=1.0)
                nc.scalar.activation(out=sexp[:], in_=sabs[:], func=Act.Exp, bias=zero_b[:R], scale=-1.0)
                nc.scalar.activation(out=sln[:], in_=sexp[:], func=Act.Ln, bias=one_b[:R], scale=1.0)
                nc.scalar.activation(out=srelu[:], in_=sc_all[:], func=Act.Relu, bias=zero_b[:R], scale=-1.0)
                nc.vector.tensor_tensor(out=obj_per[:], in0=sln[:], in1=srelu[:], op=Alu.add)
                # masked sums -> accs columns (plain mult + reduce; accum_out
                # variants of TS/TTR fail at runtime on this stack)
                nc.vector.tensor_tensor(out=scratch[:], in0=cls_per[:], in1=mask_all[:], op=Alu.mult)
                nc.vector.tensor_reduce(out=accs[:, 0:1], in_=scratch[:], axis=mybir.AxisListType.XY, op=Alu.add)
                nc.vector.tensor_tensor(out=box_per[:], in0=box_per[:], in1=mask_all[:], op=Alu.mult)
                nc.vector.tensor_reduce(out=accs[:, 1:2], in_=box_per[:], axis=mybir.AxisListType.XY, op=Alu.add)
                nc.vector.tensor_tensor(out=obj_per[:], in0=obj_per[:], in1=mask_all[:], op=Alu.mult)
                nc.vector.tensor_reduce(out=accs[:, 2:3], in_=obj_per[:], axis=mybir.AxisListType.XY, op=Alu.add)
                nc.vector.tensor_reduce(out=accs[:, 3:4], in_=mask_all[:], axis=mybir.AxisListType.XY, op=Alu.add)
            fin_ps = ps_mg.tile([4, 1], f32)
            nc.tensor.matmul(fin_ps[:], accs[:], ones_col[:R, :])
            fin_sb = s2.tile([4, 1], f32)
            nc.scalar.copy(fin_sb[:], fin_ps[:])
            nc.sync.dma_start(out=out_d[:], in_=fin_sb[:])

    nc.finalize()
    return nc


def _get_nc():
    import os
    mode = os.environ.get("KMODE", "full")
    if "nc" not in _CACHE:
        _CACHE["nc"] = _build(mode)
    return _CACHE["nc"]


def _make_in_maps(pred_boxes, pred_scores, pred_classes, gt_boxes, gt_classes):
    in_maps = []
    for c in range(NCORES):
        sl = slice(c * BL, (c + 1) * BL)
        in_maps.append({
            "pb": np.ascontiguousarray(pred_boxes[sl]).reshape(BF, R, T, 4),
            "sc": np.ascontiguousarray(pred_scores[sl]).reshape(BF, R, T),
            "pc": np.ascontiguousarray(pred_classes[sl]).reshape(BF, R, T),
            "gtb": np.ascontiguousarray(gt_boxes[sl]).reshape(BF, G, 4),
            "gtc": np.ascontiguousarray(gt_classes[sl]).reshape(BF, G),
        })
    return in_maps


def _combine(partials):
    tot = np.zeros(4, dtype=np.float32)
    for p in partials:
        tot += p.reshape(4).astype(np.float32)
    cls_s, box_s, obj_s, n = tot
    denom = np.float32(max(n, 1.0))
    if n > 0:
        cls_l = np.float32(cls_s / denom)
        box_l = np.float32(box_s / denom)
        obj_l = np.float32(obj_s / denom)
    else:
        cls_l = box_l = obj_l = np.float32(0.0)
    loss = np.float32(cls_l + box_l + obj_l)
    return np.stack([loss, cls_l, box_l, obj_l]).astype(np.float32)


def kernel(pred_boxes, pred_scores, pred_classes, gt_boxes, gt_classes):
    from concourse.bass_utils import run_bass_kernel_spmd

    nc = _get_nc()
    in_maps = _make_in_maps(pred_boxes, pred_scores, pred_classes, gt_boxes, gt_classes)
    res = run_bass_kernel_spmd(nc, in_maps, list(range(NCORES)))
    return _combine([res.results[c]["partials"] for c in range(NCORES)])
